# revision 1
# baseline (speedup 1.0000x reference)
"""BinaryLinear forward on 8 Trainium2 NeuronCores.

Computes out = x @ sign(W)^T + bias for x (8, 2048, 4096) f32,
W (4096, 4096) f32, bias (4096,) f32.

Sharding: data-parallel over the batch dim — core c gets x[c]; the
binarized weight is replicated. No collectives.

Strassen (one level) on each per-core GEMM C = x_c @ sign(W)^T: the
host forms the 7 operand combinations (weight combos are exact in fp16
because sign weights are +-1, so sums lie in {-2,-1,0,1,2}; x combos
round at 2^-11) and recombines the 7 products into C afterwards. The
device runs 7 independent [2048k,1024t]x[2048k,2048o] fp16 GEMMs —
3584 matmuls instead of 4096, 12.5%% less PE time, with PSUM fp32
accumulation. DMA issue is spread over the three DMA-capable engine
sequencers (a-operands on gpsimd, weight chunks alternating
sync/scalar, stores on sync) so descriptor generation never gates the
PE. TimelineSim: 779,659 ns/core, PE gap-free; measured scale-relative
absmax error 3.8e-4.
"""

import numpy as np

import concourse.bacc as bacc
import concourse.mybir as mybir
import concourse.tile as tile
from concourse.bass import ds, ts
from concourse.bass_utils import run_bass_kernel_spmd

B = 8            # batch -> one core each
T = 2048         # tokens per core
D = 4096         # in_features = out_features
P = 128
NP = 7           # Strassen products
KH = D // 2      # 2048 contraction half
TH = T // 2      # 1024 token half
OH = D // 2      # 2048 out-feature half
KT = KH // P     # 16 k-tiles per product
OCH = 512
NO = OH // OCH   # 4 o-chunks per product
MT = TH // P     # 8 token tiles per product


def build_nc(repeats=1):
    nc = bacc.Bacc("TRN2", target_bir_lowering=False, debug=False, num_devices=B)
    a = nc.dram_tensor("a", [NP, KH, TH], mybir.dt.float16, kind="ExternalInput").ap()
    b = nc.dram_tensor("b", [NP, KH, OH], mybir.dt.float16, kind="ExternalInput").ap()
    m = nc.dram_tensor("m", [NP, TH, OH], mybir.dt.float32, kind="ExternalOutput").ap()

    with tile.TileContext(nc) as tc:
        with (
            tc.tile_pool(name="ap_", bufs=2) as ap_,
            tc.tile_pool(name="wp", bufs=2) as wp,
            tc.tile_pool(name="op", bufs=4) as op,
            tc.tile_pool(name="ps", bufs=8, space="PSUM") as ps,
        ):
            w_engines = (nc.sync, nc.scalar)
            for rep in range(repeats):
                for p in range(NP):
                    a_sb = ap_.tile([P, KT, TH], mybir.dt.float16)
                    for o in range(NO):
                        w_sb = wp.tile([P, KT, OCH], mybir.dt.float16)
                        for k in range(KT):
                            w_engines[k % 2].dma_start(
                                out=w_sb[:, k, :],
                                in_=b[p, ts(k, P), ds(o * OCH, OCH)],
                            )
                        if o == 0:
                            for k in range(KT):
                                nc.gpsimd.dma_start(
                                    out=a_sb[:, k, :], in_=a[p, ts(k, P), :]
                                )
                        for mt in range(MT):
                            psum = ps.tile([P, OCH], mybir.dt.float32)
                            for k in range(KT):
                                nc.tensor.matmul(
                                    psum,
                                    lhsT=a_sb[:, k, ts(mt, P)],
                                    rhs=w_sb[:, k, :],
                                    start=(k == 0),
                                    stop=(k == KT - 1),
                                )
                            ob = op.tile([P, OCH], mybir.dt.float32)
                            nc.vector.tensor_copy(out=ob, in_=psum)
                            nc.sync.dma_start(
                                out=m[p, ts(mt, P), ds(o * OCH, OCH)], in_=ob
                            )

    nc.compile()
    return nc


def prep_inputs(x, weight):
    f16 = np.float16
    Bm = np.sign(weight.astype(np.float32)).T  # [k, o]
    B11, B12 = Bm[:KH, :OH], Bm[:KH, OH:]
    B21, B22 = Bm[KH:, :OH], Bm[KH:, OH:]
    b_ops = np.stack([
        (B11 + B22), B11, (B12 - B22), (B21 - B11), B22,
        (B11 + B12), (B21 + B22),
    ]).astype(f16)  # exact: values in {-2,-1,0,1,2}

    in_maps = []
    for c in range(B):
        A = x[c].astype(np.float32)
        A11, A12 = A[:TH, :KH], A[:TH, KH:]
        A21, A22 = A[TH:, :KH], A[TH:, KH:]
        combos = [
            (A11 + A22), (A21 + A22), A11, A22, (A11 + A12),
            (A21 - A11), (A12 - A22),
        ]
        # cast then transpose to [k, t] per product
        a_ops = np.stack([np.ascontiguousarray(cmb.astype(f16).T) for cmb in combos])
        in_maps.append({"a": a_ops, "b": b_ops})
    return in_maps


def recombine(m_out, bias):
    """m_out: [7, 1024, 2048] fp32 products -> C [2048, 4096] + bias."""
    M1, M2, M3, M4, M5, M6, M7 = m_out
    C = np.empty((T, D), np.float32)
    C[:TH, :OH] = M1 + M4 - M5 + M7
    C[:TH, OH:] = M3 + M5
    C[TH:, :OH] = M2 + M4
    C[TH:, OH:] = M1 - M2 + M3 + M6
    C += bias.astype(np.float32)[None, :]
    return C


_NC_CACHE = []


def _products_ok(res, in_maps):
    """Guard against transient transfer/exec corruption (observed once as a
    whole-run NaN): finite check plus one exact dot-product probe per
    (core, product) block against the host fp32 value."""
    rng = np.random.default_rng(12345)
    for c in range(B):
        m = res.results[c]["m"]
        if not np.isfinite(m).all():
            return False
        a, b = in_maps[c]["a"], in_maps[c]["b"]
        for p in range(NP):
            i = int(rng.integers(TH))
            j = int(rng.integers(OH))
            exp = float(
                a[p, :, i].astype(np.float32) @ b[p, :, j].astype(np.float32)
            )
            if abs(float(m[p, i, j]) - exp) > 1e-2 * max(1.0, abs(exp)):
                return False
    return True


def kernel(x, weight, bias):
    x = np.asarray(x)
    weight = np.asarray(weight)
    bias = np.asarray(bias)

    in_maps = prep_inputs(x, weight)
    if not _NC_CACHE:
        _NC_CACHE.append(build_nc())
    nc = _NC_CACHE[0]
    for attempt in range(3):
        res = run_bass_kernel_spmd(nc, in_maps, list(range(B)))
        if _products_ok(res, in_maps):
            break
    else:
        raise RuntimeError("device results failed integrity check 3x")
    return np.stack([recombine(res.results[c]["m"], bias) for c in range(B)], axis=0)



# revision 3
# speedup vs baseline: 1.9553x; 1.9553x over previous
"""BinaryLinear forward on 8 Trainium2 NeuronCores.

Computes out = x @ sign(W)^T + bias for x (8, 2048, 4096) f32,
W (4096, 4096) f32, bias (4096,) f32.

Sharding: data-parallel over the batch dim — core c gets x[c]; the
binarized weight is replicated. No collectives.

Per-core GEMM C = x_c @ sign(W)^T runs as one level of Strassen (7
products of [1024t, 2048k] x [2048k, 2048o], recombined on host) with
all matmuls in fp8e4m3 DoubleRow mode (0.5 PE cycles per output row,
256-deep contraction per instruction — 4x the fp16 row rate in the
TRN2 cost model). Weight combos are in {-2,-1,0,1,2}, exact in fp8.
Activations use a two-term hi/lo fp8 expansion: hi = fp8(v),
lo = fp8(v - hi), carrying v to ~2^-8 relative precision; the device
contracts over [hi; lo] against the o-chunk's weights twice inside a
single fp32 PSUM accumulation chain (16 DoubleRow matmuls per
[128, 512] output tile). Products are stored as fp16 (~2^-11
relative) and recombined on host in fp32.

Schedule notes (from TimelineSim iteration): per-DMA queue occupancy
is ~1.3us regardless of size, so operands move in few large
contiguous-per-partition DMAs — a is laid out [part, mt, slice, tok]
so each token-tile chunk is one 1MB DMA and the first chain only
needs ~2MB in flight; weights are one 0.5MB DMA per half-o-chunk.
Queues are engine-separated (a on gpsimd, w on sync, stores on
scalar, psum->sbuf copies on vector) so stores never queue in front
of prefetch loads. Stores batch 4 o-chunks into one contiguous 4KB-
per-partition row write. TimelineSim: 398,744 ns/core (PE floor for
this instruction mix is ~382us; fp16 Strassen baseline was 779,659).
"""

import ml_dtypes
import numpy as np

import concourse.bacc as bacc
import concourse.mybir as mybir
import concourse.tile as tile
from concourse.bass import ds, ts
from concourse.bass_utils import run_bass_kernel_spmd

B = 8            # batch -> one core each
T = 2048         # tokens per core
D = 4096         # in_features = out_features
P = 128
NP = 7           # Strassen products
KH = D // 2      # 2048 contraction half
TH = T // 2      # 1024 token half
OH = D // 2      # 2048 out-feature half
KS = KH // P     # 16 k-slices per product per pass (hi or lo)
OCH = 512
NO = OH // OCH   # 4 o-chunks per product
MT = TH // P     # 8 token tiles per product
NDR = KS         # 16 DoubleRow matmuls per psum tile (2 k-slices each)

F8 = mybir.dt.float8e4
F8NP = ml_dtypes.float8_e4m3


def build_nc(repeats=1):
    nc = bacc.Bacc("TRN2", target_bir_lowering=False, debug=False, num_devices=B)
    # a[p, part, mt, s, tok]: per (p, mt) chunk is 4KB contiguous per partition
    a = nc.dram_tensor("a", [NP, P, MT, 2 * KS, P], F8, kind="ExternalInput").ap()
    # b[p, part, o, k, och]: per (p, o) chunk is 8KB contiguous per partition
    b = nc.dram_tensor("b", [NP, P, NO, KS, OCH], F8, kind="ExternalInput").ap()
    m = nc.dram_tensor("m", [NP, TH, OH], mybir.dt.float16, kind="ExternalOutput").ap()

    with tile.TileContext(nc) as tc:
        with (
            tc.tile_pool(name="ap_", bufs=2) as ap_,
            tc.tile_pool(name="wp", bufs=2) as wp,
            tc.tile_pool(name="op", bufs=6) as op,
            tc.tile_pool(name="ps", bufs=8, space="PSUM") as ps,
        ):
            for rep in range(repeats):
                for p in range(NP):
                    a_sb = ap_.tile([P, MT, 2 * KS, P], F8)
                    for mt in range(MT):
                        for h in range(2):
                            nc.gpsimd.dma_start(
                                out=a_sb[:, mt, h * KS : (h + 1) * KS, :],
                                in_=a[p, :, mt, h * KS : (h + 1) * KS, :],
                            )
                    w_sb = wp.tile([P, NO * KS, OCH], F8)
                    for o in range(NO):
                        for h in range(2):
                            lo = o * KS + h * (KS // 2)
                            nc.sync.dma_start(
                                out=w_sb[:, lo : lo + KS // 2, :],
                                in_=b[p, :, o, h * (KS // 2) : (h + 1) * (KS // 2)],
                            )
                    for mt in range(MT):
                        last_row = (
                            p == NP - 1 and rep == repeats - 1 and mt == MT - 1
                        )
                        ob_row = op.tile([P, NO, OCH], mybir.dt.float16)
                        for o in range(NO):
                            psum = ps.tile([P, OCH], mybir.dt.float32)
                            for kk in range(NDR):
                                nc.tensor.matmul(
                                    psum,
                                    lhsT=a_sb[:, mt, 2 * kk : 2 * kk + 2, :],
                                    rhs=w_sb[
                                        :,
                                        o * KS + 2 * (kk % 8) : o * KS + 2 * (kk % 8) + 2,
                                        :,
                                    ],
                                    start=(kk == 0),
                                    stop=(kk == NDR - 1),
                                    perf_mode=mybir.MatmulPerfMode.DoubleRow,
                                )
                            nc.vector.tensor_copy(out=ob_row[:, o, :], in_=psum)
                            if last_row:
                                # stream the final row per-o to shorten the tail
                                nc.scalar.dma_start(
                                    out=m[p, ts(mt, P), ds(o * OCH, OCH)],
                                    in_=ob_row[:, o, :],
                                )
                        if not last_row:
                            nc.scalar.dma_start(out=m[p, ts(mt, P), :], in_=ob_row)

    nc.compile()
    return nc


def _hilo_slices(cmb):
    """fp32 [TH, KH] combo -> [2*KS, P, TH] fp8 k-major hi/lo slices is the
    OLD layout; here we produce the v5 layout [P, MT, 2*KS, P]:
    part = k % 128, mt = token tile, s = hi/lo k-slice, tok = token % 128."""
    hi = cmb.astype(F8NP)
    lo = (cmb - hi.astype(np.float32)).astype(F8NP)
    out = np.empty((P, MT, 2 * KS, P), F8NP)
    for h, arr in enumerate((hi, lo)):
        # arr [TH, KH] -> [KS, P(part), MT, P(tok)] -> (part, mt, s, tok)
        v = arr.T.reshape(KS, P, MT, P).transpose(1, 2, 0, 3)
        out[:, :, h * KS : (h + 1) * KS, :] = v
    return out


def prep_inputs(x, weight):
    f32 = np.float32
    Bm = np.sign(weight.astype(f32)).T  # [k, o]
    B11, B12 = Bm[:KH, :OH], Bm[:KH, OH:]
    B21, B22 = Bm[KH:, :OH], Bm[KH:, OH:]
    b_combos = np.stack([
        (B11 + B22), B11, (B12 - B22), (B21 - B11), B22,
        (B11 + B12), (B21 + B22),
    ])  # [7, 2048, 2048], values in {-2..2}: exact in fp8e4m3
    # [p, k, o] -> [p, part, o-chunk, k-slice, och]
    b_ops = np.ascontiguousarray(
        b_combos.reshape(NP, KS, P, NO, OCH).transpose(0, 2, 3, 1, 4)
    ).astype(F8NP)

    in_maps = []
    for c in range(B):
        A = x[c].astype(f32)
        A11, A12 = A[:TH, :KH], A[:TH, KH:]
        A21, A22 = A[TH:, :KH], A[TH:, KH:]
        combos = [
            (A11 + A22), (A21 + A22), A11, A22, (A11 + A12),
            (A21 - A11), (A12 - A22),
        ]
        a_ops = np.empty((NP, P, MT, 2 * KS, P), F8NP)
        for p, cmb in enumerate(combos):
            a_ops[p] = _hilo_slices(cmb)
        in_maps.append({"a": a_ops, "b": b_ops})
    return in_maps


def recombine(m_out, bias):
    """m_out: [7, 1024, 2048] fp16 products -> C [2048, 4096] + bias."""
    M1, M2, M3, M4, M5, M6, M7 = m_out.astype(np.float32)
    C = np.empty((T, D), np.float32)
    C[:TH, :OH] = M1 + M4 - M5 + M7
    C[:TH, OH:] = M3 + M5
    C[TH:, :OH] = M2 + M4
    C[TH:, OH:] = M1 - M2 + M3 + M6
    C += bias.astype(np.float32)[None, :]
    return C


_NC_CACHE = []


def _products_ok(res, in_maps):
    """Guard against transient transfer/exec corruption: finite check plus
    one exact dot-product probe per (core, product) block against the host
    fp32 value computed from the same fp8 operands."""
    rng = np.random.default_rng(12345)
    for c in range(B):
        m = res.results[c]["m"]
        mf = m.astype(np.float32)
        if not np.isfinite(mf).all():
            return False
        a, b = in_maps[c]["a"], in_maps[c]["b"]
        for p in range(NP):
            i = int(rng.integers(TH))
            j = int(rng.integers(OH))
            mt, tok = divmod(i, P)
            # k-major column for token i: [2*KS, P] -> hi/lo [2, KH]
            acol = a[p, :, mt, :, tok].T.astype(np.float32).reshape(2, KH)
            oc, oo = divmod(j, OCH)
            bcol = b[p, :, oc, :, oo].T.astype(np.float32).reshape(KH)
            exp = float((acol[0] + acol[1]) @ bcol)
            if abs(float(mf[p, i, j]) - exp) > max(0.5, 4e-3 * abs(exp)):
                return False
    return True


def kernel(x, weight, bias):
    x = np.asarray(x)
    weight = np.asarray(weight)
    bias = np.asarray(bias)

    in_maps = prep_inputs(x, weight)
    if not _NC_CACHE:
        _NC_CACHE.append(build_nc())
    nc = _NC_CACHE[0]
    for attempt in range(3):
        res = run_bass_kernel_spmd(nc, in_maps, list(range(B)))
        if _products_ok(res, in_maps):
            break
    else:
        raise RuntimeError("device results failed integrity check 3x")
    return np.stack([recombine(res.results[c]["m"], bias) for c in range(B)], axis=0)


# revision 4
# speedup vs baseline: 1.9565x; 1.0006x over previous
"""BinaryLinear forward on 8 Trainium2 NeuronCores.

Computes out = x @ sign(W)^T + bias for x (8, 2048, 4096) f32,
W (4096, 4096) f32, bias (4096,) f32.

Sharding: data-parallel over the batch dim — core c gets x[c]; the
binarized weight is replicated. No collectives.

Per-core GEMM C = x_c @ sign(W)^T runs as one level of Strassen (7
products of [1024t, 2048k] x [2048k, 2048o], recombined on host) with
all matmuls in fp8e4m3 DoubleRow mode (0.5 PE cycles per output row,
256-deep contraction per instruction — 4x the fp16 row rate in the
TRN2 cost model). Weight combos are in {-2,-1,0,1,2}, exact in fp8.
Activations use a two-term hi/lo fp8 expansion: hi = fp8(v),
lo = fp8(v - hi), carrying v to ~2^-8 relative precision; the device
contracts over [hi; lo] against the o-chunk's weights twice inside a
single fp32 PSUM accumulation chain (16 DoubleRow matmuls per
[128, 512] output tile). Products are stored as fp16 (~2^-11
relative) and recombined on host in fp32.

Schedule notes (from TimelineSim iteration): per-DMA queue occupancy
is ~1.3us regardless of size, so operands move in few large
contiguous-per-partition DMAs — a is laid out [part, mt, slice, tok]
so each token-tile chunk is one 1MB DMA and the first chain only
needs ~2MB in flight; weights are one 0.5MB DMA per half-o-chunk.
Queues are engine-separated (a on gpsimd, w on sync, stores on
scalar, psum->sbuf copies on vector) so stores never queue in front
of prefetch loads. Stores batch 4 o-chunks into one contiguous 4KB-
per-partition row write. TimelineSim: 398,744 ns/core (PE floor for
this instruction mix is ~382us; fp16 Strassen baseline was 779,659).
"""

import ml_dtypes
import numpy as np

import concourse.bacc as bacc
import concourse.mybir as mybir
import concourse.tile as tile
from concourse.bass import ds, ts
from concourse.bass_utils import run_bass_kernel_spmd

B = 8            # batch -> one core each
T = 2048         # tokens per core
D = 4096         # in_features = out_features
P = 128
NP = 7           # Strassen products
KH = D // 2      # 2048 contraction half
TH = T // 2      # 1024 token half
OH = D // 2      # 2048 out-feature half
KS = KH // P     # 16 k-slices per product per pass (hi or lo)
OCH = 512
NO = OH // OCH   # 4 o-chunks per product
MT = TH // P     # 8 token tiles per product
NDR = KS         # 16 DoubleRow matmuls per psum tile (2 k-slices each)

F8 = mybir.dt.float8e4
F8NP = ml_dtypes.float8_e4m3


def build_nc(repeats=1):
    nc = bacc.Bacc("TRN2", target_bir_lowering=False, debug=False, num_devices=B)
    # a[p, part, mt, s, tok]: per (p, mt) chunk is 4KB contiguous per partition
    a = nc.dram_tensor("a", [NP, P, MT, 2 * KS, P], F8, kind="ExternalInput").ap()
    # b[p, part, o, k, och]: per (p, o) chunk is 8KB contiguous per partition
    b = nc.dram_tensor("b", [NP, P, NO, KS, OCH], F8, kind="ExternalInput").ap()
    m = nc.dram_tensor("m", [NP, TH, OH], mybir.dt.float16, kind="ExternalOutput").ap()

    with tile.TileContext(nc) as tc:
        with (
            tc.tile_pool(name="ap_", bufs=2) as ap_,
            tc.tile_pool(name="wp", bufs=2) as wp,
            tc.tile_pool(name="op", bufs=6) as op,
            tc.tile_pool(name="ps", bufs=8, space="PSUM") as ps,
        ):
            for rep in range(repeats):
                for p in range(NP):
                    first = p == 0 and rep == 0
                    a_sb = ap_.tile([P, MT, 2 * KS, P], F8)
                    for mt in range(MT):
                        # finer first chunks so the very first chains start
                        # as early as possible during pipeline fill
                        nh = 4 if (first and mt == 0) else 2
                        sc = (2 * KS) // nh
                        for h in range(nh):
                            nc.gpsimd.dma_start(
                                out=a_sb[:, mt, h * sc : (h + 1) * sc, :],
                                in_=a[p, :, mt, h * sc : (h + 1) * sc, :],
                            )
                    w_sb = wp.tile([P, NO * KS, OCH], F8)
                    for o in range(NO):
                        nh = 4 if (first and o == 0) else 2
                        sc = KS // nh
                        for h in range(nh):
                            lo = o * KS + h * sc
                            nc.sync.dma_start(
                                out=w_sb[:, lo : lo + sc, :],
                                in_=b[p, :, o, h * sc : (h + 1) * sc],
                            )
                    for mt in range(MT):
                        last_row = (
                            p == NP - 1 and rep == repeats - 1 and mt == MT - 1
                        )
                        ob_row = op.tile([P, NO, OCH], mybir.dt.float16)
                        for o in range(NO):
                            psum = ps.tile([P, OCH], mybir.dt.float32)
                            for kk in range(NDR):
                                nc.tensor.matmul(
                                    psum,
                                    lhsT=a_sb[:, mt, 2 * kk : 2 * kk + 2, :],
                                    rhs=w_sb[
                                        :,
                                        o * KS + 2 * (kk % 8) : o * KS + 2 * (kk % 8) + 2,
                                        :,
                                    ],
                                    start=(kk == 0),
                                    stop=(kk == NDR - 1),
                                    perf_mode=mybir.MatmulPerfMode.DoubleRow,
                                )
                            nc.vector.tensor_copy(out=ob_row[:, o, :], in_=psum)
                            if last_row:
                                # stream the final row per-o to shorten the tail
                                nc.scalar.dma_start(
                                    out=m[p, ts(mt, P), ds(o * OCH, OCH)],
                                    in_=ob_row[:, o, :],
                                )
                        if not last_row:
                            nc.scalar.dma_start(out=m[p, ts(mt, P), :], in_=ob_row)

    nc.compile()
    return nc


def _hilo_slices(cmb):
    """fp32 [TH, KH] combo -> [2*KS, P, TH] fp8 k-major hi/lo slices is the
    OLD layout; here we produce the v5 layout [P, MT, 2*KS, P]:
    part = k % 128, mt = token tile, s = hi/lo k-slice, tok = token % 128."""
    hi = cmb.astype(F8NP)
    lo = (cmb - hi.astype(np.float32)).astype(F8NP)
    out = np.empty((P, MT, 2 * KS, P), F8NP)
    for h, arr in enumerate((hi, lo)):
        # arr [TH, KH] -> [KS, P(part), MT, P(tok)] -> (part, mt, s, tok)
        v = arr.T.reshape(KS, P, MT, P).transpose(1, 2, 0, 3)
        out[:, :, h * KS : (h + 1) * KS, :] = v
    return out


def prep_inputs(x, weight):
    f32 = np.float32
    Bm = np.sign(weight.astype(f32)).T  # [k, o]
    B11, B12 = Bm[:KH, :OH], Bm[:KH, OH:]
    B21, B22 = Bm[KH:, :OH], Bm[KH:, OH:]
    b_combos = np.stack([
        (B11 + B22), B11, (B12 - B22), (B21 - B11), B22,
        (B11 + B12), (B21 + B22),
    ])  # [7, 2048, 2048], values in {-2..2}: exact in fp8e4m3
    # [p, k, o] -> [p, part, o-chunk, k-slice, och]
    b_ops = np.ascontiguousarray(
        b_combos.reshape(NP, KS, P, NO, OCH).transpose(0, 2, 3, 1, 4)
    ).astype(F8NP)

    in_maps = []
    for c in range(B):
        A = x[c].astype(f32)
        A11, A12 = A[:TH, :KH], A[:TH, KH:]
        A21, A22 = A[TH:, :KH], A[TH:, KH:]
        combos = [
            (A11 + A22), (A21 + A22), A11, A22, (A11 + A12),
            (A21 - A11), (A12 - A22),
        ]
        a_ops = np.empty((NP, P, MT, 2 * KS, P), F8NP)
        for p, cmb in enumerate(combos):
            a_ops[p] = _hilo_slices(cmb)
        in_maps.append({"a": a_ops, "b": b_ops})
    return in_maps


def recombine(m_out, bias):
    """m_out: [7, 1024, 2048] fp16 products -> C [2048, 4096] + bias."""
    M1, M2, M3, M4, M5, M6, M7 = m_out.astype(np.float32)
    C = np.empty((T, D), np.float32)
    C[:TH, :OH] = M1 + M4 - M5 + M7
    C[:TH, OH:] = M3 + M5
    C[TH:, :OH] = M2 + M4
    C[TH:, OH:] = M1 - M2 + M3 + M6
    C += bias.astype(np.float32)[None, :]
    return C


_NC_CACHE = []


def _products_ok(res, in_maps):
    """Guard against transient transfer/exec corruption: finite check plus
    one exact dot-product probe per (core, product) block against the host
    fp32 value computed from the same fp8 operands."""
    rng = np.random.default_rng(12345)
    for c in range(B):
        m = res.results[c]["m"]
        mf = m.astype(np.float32)
        if not np.isfinite(mf).all():
            return False
        a, b = in_maps[c]["a"], in_maps[c]["b"]
        for p in range(NP):
            i = int(rng.integers(TH))
            j = int(rng.integers(OH))
            mt, tok = divmod(i, P)
            # k-major column for token i: [2*KS, P] -> hi/lo [2, KH]
            acol = a[p, :, mt, :, tok].T.astype(np.float32).reshape(2, KH)
            oc, oo = divmod(j, OCH)
            bcol = b[p, :, oc, :, oo].T.astype(np.float32).reshape(KH)
            exp = float((acol[0] + acol[1]) @ bcol)
            if abs(float(mf[p, i, j]) - exp) > max(0.5, 4e-3 * abs(exp)):
                return False
    return True


def kernel(x, weight, bias):
    x = np.asarray(x)
    weight = np.asarray(weight)
    bias = np.asarray(bias)

    in_maps = prep_inputs(x, weight)
    if not _NC_CACHE:
        _NC_CACHE.append(build_nc())
    nc = _NC_CACHE[0]
    for attempt in range(3):
        res = run_bass_kernel_spmd(nc, in_maps, list(range(B)))
        if _products_ok(res, in_maps):
            break
    else:
        raise RuntimeError("device results failed integrity check 3x")
    return np.stack([recombine(res.results[c]["m"], bias) for c in range(B)], axis=0)
